# revision 9
# baseline (speedup 1.0000x reference)
"""Trainium2 Bass kernel for nn_AttentionModel_6468220748059.

Self-contained: host-folds BN/conv weights + attention weight stacks,
shards batch B=32 across 8 NeuronCores (4 per core), runs a Tile kernel.

Distance-weighted attention trick: with W=|s-t|/n, c=E^-0.5,
  A[t,s] = sum_d khat[t,d] q[s,d] - k[t,d] qhat[s,d]  (khat=(t/n)k, qhat=(s/n)q)
         = ((t-s)/n) * (q_s . k_t)
so exp(c*W*(q.k)) = exp(+c*A) for t>=s, exp(-c*A) for t<s — exp's per-tile
scale handles the W multiply for free; only diagonal 128x128 blocks need a
DVE sign-fix. Softmax denom comes from an appended ones column in V.
"""

import numpy as np

B, L, S, E, H, NCLS = 32, 4100, 1024, 16, 4, 10
DH = E // H
EPS = 1e-5
CSCALE = float(E) ** -0.5
NB = 4  # batches per core
NCORES = 8
TT = 128


# ----------------------------------------------------------------- host prep
def host_prep(inputs):
    f32 = np.float32
    p = {}
    inp = {k: np.asarray(v, dtype=f32) for k, v in inputs.items()}
    bnscale = 1.0 / np.sqrt(1.0 + EPS)

    s1 = inp["bn1_g"] * bnscale
    w1 = inp["patch_w"][:, 0, :] * s1[:, None]
    p["w1T"] = np.ascontiguousarray(w1.T)  # [8k, 8c]
    p["b1"] = (inp["patch_b"] * s1 + inp["bn1_b"]).reshape(8, 1).astype(f32)

    s2 = inp["bn2_g"] * bnscale
    w2 = inp["emb_w"] * s2[:, None, None]
    w2T = np.zeros((64, 32), f32)
    for k in range(8):
        for ci in range(8):
            w2T[k * 8 + ci, :] = w2[:, ci, k]
    p["w2T"] = w2T
    p["b2"] = (inp["emb_b"] * s2 + inp["bn2_b"]).reshape(32, 1).astype(f32)

    s3 = inp["bn3_g"] * bnscale
    dw, pw = inp["dw_w"], inp["pw_w"][:, :, 0]
    comb = np.zeros((32, 32, 16), f32)
    for k in range(32):
        for m in range(16):
            for j in range(2):
                comb[k, 2 * m + j, :] = s3 * pw[:, m] * dw[m, j, k]
    w3T = np.zeros((8, 128, 16), f32)
    for g in range(8):
        for kk in range(4):
            w3T[g, kk * 32 : kk * 32 + 32, :] = comb[4 * g + kk]
    p["w3T"] = np.ascontiguousarray(w3T.transpose(1, 0, 2))  # [128, 8, 16]
    p["b3"] = inp["bn3_b"].reshape(16, 1).astype(f32)

    pos = np.arange(S, dtype=f32)[:, None]
    div = np.exp(np.arange(0, E, 2, dtype=f32) * (-np.log(10000.0) / E))
    ang = pos * div[None, :] * (E / S)
    pe = np.zeros((S, E), f32)
    pe[:, 0::2] = np.sin(ang)
    pe[:, 1::2] = np.cos(ang)
    p["peT"] = np.ascontiguousarray(pe.T)  # [16, S]
    pf = np.zeros((128, 128), f32)
    for j in range(8):
        pf[:, j * 16 : (j + 1) * 16] = pe[128 * j : 128 * (j + 1), :]
    p["pe_fold"] = pf

    for pref in ("1", "2"):
        wq, wk, wv = inp[f"q{pref}_w"], inp[f"k{pref}_w"], inp[f"v{pref}_w"]
        WqA = np.zeros((16, 4, 32), f32)
        WqB = np.zeros((16, 4, 32), f32)
        WkA = np.zeros((16, 4, 32), f32)
        WkB = np.zeros((16, 4, 32), f32)
        for h in range(4):
            WqA[:, h, 0:4] = wq[4 * h : 4 * h + 4, :].T
            WqB[:, h, 4:8] = wq[4 * h : 4 * h + 4, :].T
            WkB[:, h, 0:4] = -wk[4 * h : 4 * h + 4, :].T
            WkA[:, h, 4:8] = wk[4 * h : 4 * h + 4, :].T
        p[f"Wq{pref}A"] = WqA
        p[f"Wq{pref}B"] = WqB
        p[f"Wk{pref}A"] = WkA
        p[f"Wk{pref}B"] = WkB
        Wv = np.zeros((17, 128), f32)
        for h in range(4):
            Wv[0:16, 32 * h : 32 * h + 4] = wv[4 * h : 4 * h + 4, :].T
            Wv[16, 32 * h + 4] = 1.0
        p[f"Wv{pref}"] = Wv

    p["sgnmask"] = np.sign(
        np.arange(TT, dtype=f32)[:, None] - np.arange(TT, dtype=f32)[None, :]
    ).astype(f32)
    sv = np.zeros((16, S), f32)
    sv[:, :] = -(np.arange(S, dtype=f32) / S)[None, :]
    p["svecneg"] = sv
    p["ones_row"] = np.ones((1, S), f32)
    sel = np.zeros((128, 20), f32)
    for h in range(4):
        for j in range(5):
            sel[32 * h + j, 5 * h + j] = 1.0
    p["selT"] = sel
    p["identity"] = np.eye(128, dtype=f32)
    p["ones_col"] = np.ones((128, 1), f32)
    for nm in ("lna1", "ln1", "lna2", "ln2"):
        p[f"{nm}_g"] = np.broadcast_to(inp[f"{nm}_g"], (128, 16)).copy()
        p[f"{nm}_b"] = np.broadcast_to(inp[f"{nm}_b"], (128, 16)).copy()
    selE = np.zeros((128, 16), f32)
    for j in range(8):
        for e in range(16):
            selE[16 * j + e, e] = 1.0 / S
    p["selE"] = selE
    p["woutT"] = np.ascontiguousarray(inp["out_w"].T)
    p["bout"] = inp["out_b"].reshape(1, NCLS).astype(f32)

    x = inp["x"][:, 0, :]
    x8 = np.zeros((B, 8, S), f32)
    for k in range(8):
        x8[:, k, :] = x[:, k : k + 4 * S : 4][:, :S]
    p["x8"] = x8

    # which LN affine transforms are trivial (skip ops)
    p["_ln_trivial"] = {
        nm: bool(
            np.allclose(inp[f"{nm}_g"], 1.0) and np.allclose(inp[f"{nm}_b"], 0.0)
        )
        for nm in ("lna1", "ln1", "lna2", "ln2")
    }
    return p


# ------------------------------------------------------------- kernel build
_BUILD_CACHE = {}

CONST_SPECS = [
    # name, shape, dtype tag: False=f32, True=bf16, "r"=float32r
    ("w1T", (8, 8), True),
    ("b1", (8, 1), False),
    ("w2T", (64, 32), True),
    ("b2", (32, 1), False),
    ("w3T", (128, 8, 16), True),
    ("b3", (16, 1), False),
    ("peT", (16, S), False),
    ("pe_fold", (128, 128), False),
    ("Wq1A", (16, 4, 32), True),
    ("Wq1B", (16, 4, 32), True),
    ("Wk1A", (16, 4, 32), True),
    ("Wk1B", (16, 4, 32), True),
    ("Wv1", (17, 128), True),
    ("Wq2A", (16, 4, 32), True),
    ("Wq2B", (16, 4, 32), True),
    ("Wk2A", (16, 4, 32), True),
    ("Wk2B", (16, 4, 32), True),
    ("Wv2", (17, 128), True),
    ("sgnmask", (128, 128), False),
    ("svecneg", (16, S), True),
    ("ones_row", (1, S), True),
    ("selT", (128, 20), False),
    ("identity", (128, 128), False),
    ("ones_col", (128, 1), False),
    ("lna1_g", (128, 16), False),
    ("lna1_b", (128, 16), False),
    ("ln1_g", (128, 16), False),
    ("ln1_b", (128, 16), False),
    ("lna2_g", (128, 16), False),
    ("lna2_b", (128, 16), False),
    ("ln2_g", (128, 16), False),
    ("ln2_b", (128, 16), False),
    ("selE", (128, 16), False),
    ("woutT", (16, NCLS), False),
    ("bout", (1, NCLS), False),
]


def _brd(ap, count):
    """Append a broadcast (step 0) innermost free dim to an AP."""
    import concourse.bass as bass

    return bass.AP(tensor=ap.tensor, offset=ap.offset, ap=[*list(ap.ap), [0, count]])


def build_nc(ln_trivial):
    import concourse.bass as bass
    import concourse.bacc as bacc
    import concourse.tile as tile
    from concourse import mybir

    f32 = mybir.dt.float32
    f32r = mybir.dt.float32r
    bf16 = mybir.dt.bfloat16
    ALU = mybir.AluOpType
    AF = mybir.ActivationFunctionType

    nc = bacc.Bacc(trn_type="TRN2", target_bir_lowering=False, debug=False)

    dram = {}
    dt_of = {False: f32, True: bf16, "r": f32r}
    for name, shape, isbf in CONST_SPECS:
        dram[name] = nc.dram_tensor(
            name, list(shape), dt_of[isbf], kind="ExternalInput"
        ).ap()
    dram["x8"] = nc.dram_tensor("x8", [NB, 8, S], bf16, kind="ExternalInput").ap()
    out_d = nc.dram_tensor("out", [NB, NCLS], f32, kind="ExternalOutput").ap()

    with tile.TileContext(nc) as tc:
        import contextlib

        ctx = contextlib.ExitStack()
        cpool = ctx.enter_context(tc.tile_pool(name="consts", bufs=1))
        perb = ctx.enter_context(tc.tile_pool(name="perb", bufs=NB))
        work = ctx.enter_context(tc.tile_pool(name="work", bufs=2))
        wexp = ctx.enter_context(tc.tile_pool(name="wexp", bufs=3))
        small = ctx.enter_context(tc.tile_pool(name="small", bufs=4))
        pp_s = ctx.enter_context(tc.tile_pool(name="pp_s", bufs=1, space="PSUM"))
        pp_av = ctx.enter_context(tc.tile_pool(name="pp_av", bufs=2, space="PSUM"))
        pp_sm = ctx.enter_context(tc.tile_pool(name="pp_sm", bufs=2, space="PSUM"))

        # ---- load constants
        C = {}
        for name, shape, isbf in CONST_SPECS:
            t = cpool.tile(list(shape), dt_of[isbf], name=f"c_{name}")
            nc.sync.dma_start(out=t, in_=dram[name])
            C[name] = t

        h4_b = []        # [16, S] f32 per batch (x_srcT)
        xsrc_fold_b = [] # [128, 128] f32
        att_fold_b = [None] * NB

        eps_sb = cpool.tile([128, 1], f32, name="eps_sb")
        nc.vector.memset(eps_sb, EPS)

        def ln_fold(src, nm, extra_add=None):
            """LN over e-groups of 16 in folded [128, (j,16)] layout.
            src: [128,128] f32 SBUF tile. Returns new [128,128] f32 tile.
            extra_add: optional [128,128] tile added BEFORE the LN (residual)."""
            if extra_add is not None:
                tmp = small.tile([128, 128], f32, name=f"res_{nm}", tag="lnres")
                nc.vector.tensor_add(tmp, src, extra_add)
                src = tmp
            s3d = src.rearrange("p (j e) -> p j e", e=16)
            sums = small.tile([128, 8], f32, name=f"sums_{nm}", tag="lnsum")
            nc.vector.tensor_reduce(
                out=sums, in_=s3d, axis=mybir.AxisListType.X, op=ALU.add
            )
            negmean = small.tile([128, 8], f32, name=f"nm_{nm}", tag="lnnm")
            nc.vector.tensor_scalar_mul(negmean, sums, -1.0 / 16.0)
            cen = small.tile([128, 8, 16], f32, name=f"cen_{nm}", tag="lncen")
            nc.vector.tensor_tensor(
                out=cen, in0=s3d, in1=_brd(negmean, 16), op=ALU.add
            )
            sq = small.tile([128, 8, 16], f32, name=f"sq_{nm}", tag="lnsq")
            nc.vector.tensor_mul(sq, cen, cen)
            var = small.tile([128, 8], f32, name=f"var_{nm}", tag="lnvar")
            nc.vector.tensor_reduce(
                out=var, in_=sq, axis=mybir.AxisListType.X, op=ALU.add
            )
            # rstd = exp(-0.5 * ln(var/16 + eps))  (stays in the exp/ln table set)
            lnv = small.tile([128, 8], f32, name=f"lnv_{nm}", tag="lnlnv")
            nc.scalar.activation(
                out=lnv, in_=var, func=AF.Ln, bias=eps_sb, scale=1.0 / 16.0
            )
            rstd = small.tile([128, 8], f32, name=f"rstd_{nm}", tag="lnrstd")
            nc.scalar.activation(out=rstd, in_=lnv, func=AF.Exp, scale=-0.5)
            dst = small.tile([128, 8, 16], f32, name=f"ln_{nm}", tag="lnout")
            nc.vector.tensor_tensor(
                out=dst, in0=cen, in1=_brd(rstd, 16), op=ALU.mult
            )
            dst2 = dst.rearrange("p j e -> p (j e)")
            base = nm[:-2] if nm.endswith("_l") else nm
            if not ln_trivial[base]:
                g3 = C[f"{base}_g"].rearrange("p e -> p 1 e")
                b3 = C[f"{base}_b"].rearrange("p e -> p 1 e")
                dstg = small.tile([128, 8, 16], f32, name=f"lng_{nm}", tag="lnoutg")
                nc.vector.tensor_tensor(
                    out=dstg,
                    in0=dst,
                    in1=bass.AP(
                        tensor=g3.tensor, offset=g3.offset,
                        ap=[g3.ap[0], [0, 8], g3.ap[2]],
                    ),
                    op=ALU.mult,
                )
                dstb = small.tile([128, 8, 16], f32, name=f"lnb_{nm}", tag="lnoutb")
                nc.vector.tensor_tensor(
                    out=dstb,
                    in0=dstg,
                    in1=bass.AP(
                        tensor=b3.tensor, offset=b3.offset,
                        ap=[b3.ap[0], [0, 8], b3.ap[2]],
                    ),
                    op=ALU.add,
                )
                dst2 = dstb.rearrange("p j e -> p (j e)")
            return dst2

        # ================= per-batch pipeline =================
        for b in range(NB):
            # ---------- conv front-end ----------
            x8 = work.tile([8, S], bf16, name=f"x8_{b}", tag="x8")
            nc.sync.dma_start(out=x8, in_=dram["x8"][b])

            h1p = work.tile([8, S + 8], bf16, name=f"h1p_{b}", tag="h1p")
            nc.vector.memset(h1p[:, 0:3], 0.0)
            nc.vector.memset(h1p[:, 3 + S :], 0.0)
            for c in range(2):
                ps = pp_sm.tile([8, 512], f32, name=f"h1ps_{b}_{c}", tag="psmall")
                nc.tensor.matmul(
                    ps,
                    C["w1T"],
                    x8[:, 512 * c : 512 * (c + 1)],
                    start=True,
                    stop=True,
                )
                nc.vector.tensor_scalar(
                    out=h1p[:, 3 + 512 * c : 3 + 512 * (c + 1)],
                    in0=ps,
                    scalar1=C["b1"],
                    scalar2=0.0,
                    op0=ALU.add,
                    op1=ALU.max,
                )
            h1im = work.tile([64, S + 8], bf16, name=f"h1im_{b}", tag="h1im")
            for k in range(8):
                nc.sync.dma_start(out=h1im[8 * k : 8 * k + 8, 0:S], in_=h1p[:, k : k + S])

            h2p = work.tile([32, S + 36], bf16, name=f"h2p_{b}", tag="h2p")
            nc.vector.memset(h2p[:, 0:15], 0.0)
            nc.vector.memset(h2p[:, 15 + S :], 0.0)
            for c in range(2):
                ps = pp_sm.tile([32, 512], f32, name=f"h2ps_{b}_{c}", tag="psmall")
                nc.tensor.matmul(
                    ps,
                    C["w2T"],
                    h1im[:, 512 * c : 512 * (c + 1)],
                    start=True,
                    stop=True,
                )
                nc.vector.tensor_scalar(
                    out=h2p[:, 15 + 512 * c : 15 + 512 * (c + 1)],
                    in0=ps,
                    scalar1=C["b2"],
                    scalar2=0.0,
                    op0=ALU.add,
                    op1=ALU.max,
                )
            h2im = work.tile([128, S + 36], bf16, name=f"h2im_{b}", tag="h2im")
            for kk in range(4):
                nc.sync.dma_start(
                    out=h2im[32 * kk : 32 * kk + 32, 0 : S + 32],
                    in_=h2p[:, kk : kk + S + 32],
                )
            h4 = perb.tile([16, S], f32, name=f"h4_{b}", tag="h4")
            for c in range(2):
                ps = pp_sm.tile([16, 512], f32, name=f"h3ps_{b}_{c}", tag="psmall")
                for g in range(8):
                    nc.tensor.matmul(
                        ps,
                        C["w3T"][:, g, :],
                        h2im[:, 4 * g + 512 * c : 4 * g + 512 * c + 512],
                        start=(g == 0),
                        stop=(g == 7),
                    )
                nc.vector.tensor_scalar(
                    out=h4[:, 512 * c : 512 * (c + 1)],
                    in0=ps,
                    scalar1=C["b3"],
                    scalar2=0.0,
                    op0=ALU.add,
                    op1=ALU.max,
                )
            h4_b.append(h4)

            # x_src folded [p, (j,e)] via PE transposes
            xs_ps = pp_sm.tile([128, 128], f32, name=f"xsps_{b}", tag="psmall")
            for j in range(8):
                nc.tensor.transpose(
                    xs_ps[:, 16 * j : 16 * j + 16],
                    h4[:, 128 * j : 128 * (j + 1)],
                    C["identity"][0:16, 0:16],
                )
            xsf = perb.tile([128, 128], f32, name=f"xsf_{b}", tag="xsf")
            nc.vector.tensor_copy(xsf, xs_ps)
            xsrc_fold_b.append(xsf)

        # ---------- attention layers ----------
        for li, pref in ((0, "1"), (1, "2")):
            for b in range(NB):
                xA = work.tile([17, S], bf16, name=f"xA_{li}_{b}", tag="xA")
                xB = work.tile([16, S], bf16, name=f"xB_{li}_{b}", tag="xB")
                if li == 0:
                    nc.vector.tensor_add(xA[0:16, :], h4_b[b], C["peT"])
                else:
                    x2f = small.tile([128, 128], f32, name=f"x2f_{b}", tag="x2f")
                    nc.vector.tensor_add(x2f, att_fold_b[b], C["pe_fold"])
                    for half in range(2):
                        t2 = pp_sm.tile([16, 512], f32, name=f"t2_{b}_{half}", tag="psmall")
                        for j in range(4):
                            jj = 4 * half + j
                            nc.tensor.transpose(
                                t2[:, 128 * j : 128 * (j + 1)],
                                x2f[:, 16 * jj : 16 * jj + 16],
                                C["identity"],
                            )
                        nc.vector.tensor_copy(
                            xA[0:16, 512 * half : 512 * (half + 1)], t2
                        )
                nc.sync.dma_start(out=xA[16:17, :], in_=C["ones_row"])
                nc.sync.dma_start(out=xB, in_=xA[0:16, :])
                nc.vector.tensor_mul(xB, xB, C["svecneg"])

                # q/k projections (heads col-packed)
                qT = work.tile([128, S], bf16, name=f"qT_{li}_{b}", tag="qT")
                kT = work.tile([128, S], bf16, name=f"kT_{li}_{b}", tag="kT")
                for dst, wname in ((qT, f"Wq{pref}"), (kT, f"Wk{pref}")):
                    for c in range(2):
                        ps = pp_sm.tile([128, 512], f32, name=f"qk_{li}_{b}_{c}", tag="psmall")
                        for h in range(4):
                            nc.tensor.matmul(
                                ps[32 * h : 32 * h + 32, :],
                                C[wname + "A"][:, h, :],
                                xA[0:16, 512 * c : 512 * (c + 1)],
                                start=True,
                                stop=False,
                                tile_position=(0, 32 * h),
                            )
                            nc.tensor.matmul(
                                ps[32 * h : 32 * h + 32, :],
                                C[wname + "B"][:, h, :],
                                xB[:, 512 * c : 512 * (c + 1)],
                                start=False,
                                stop=True,
                                tile_position=(0, 32 * h),
                            )
                        nc.vector.tensor_copy(dst[:, 512 * c : 512 * (c + 1)], ps)

                # v_ext per t-tile
                v_sb = work.tile([128, 8, 128], bf16, name=f"v_{li}_{b}", tag="v")
                for t in range(8):
                    vps = pp_sm.tile([128, 128], f32, name=f"vps_{li}_{b}_{t}", tag="psmall")
                    nc.tensor.matmul(
                        vps, xA[0:17, 128 * t : 128 * (t + 1)], C[f"Wv{pref}"],
                        start=True, stop=True,
                    )
                    nc.vector.tensor_copy(v_sb[:, t, :], vps)

                # scores -> exp -> AV
                o_ps = [None, None]
                for sh in range(2):
                    o_ps[sh] = pp_av.tile([128, 512], f32, name=f"ops_{li}_{b}_{sh}", tag="ops")
                    for t in range(8):
                        sps = pp_s.tile([128, 4, 512], f32, name=f"sps_{li}_{b}_{sh}_{t}", tag="sps")
                        for h in range(4):
                            nc.tensor.matmul(
                                sps[:, h, :],
                                kT[32 * h : 32 * h + 8, 128 * t : 128 * (t + 1)],
                                qT[32 * h : 32 * h + 8, 512 * sh : 512 * (sh + 1)],
                                start=True,
                                stop=True,
                                tile_position=(32 * h, 0),
                            )
                        if t // 4 == sh:
                            lc = 128 * t - 512 * sh
                            sgn = C["sgnmask"]
                            nc.vector.tensor_tensor(
                                out=sps[:, :, lc : lc + 128],
                                in0=sps[:, :, lc : lc + 128],
                                in1=bass.AP(
                                    tensor=sgn.tensor,
                                    offset=sgn.offset,
                                    ap=[sgn.ap[0], [0, 4], sgn.ap[1]],
                                ),
                                op=ALU.mult,
                            )
                        aT = wexp.tile([128, 4, 512], bf16, name=f"aT_{li}_{b}_{sh}_{t}", tag="aT")
                        bnd = min(max(128 * (t + 1) - 512 * sh, 0), 512)
                        if bnd > 0:
                            nc.scalar.activation(
                                out=aT[:, :, 0:bnd], in_=sps[:, :, 0:bnd],
                                func=AF.Exp, scale=CSCALE,
                            )
                        if bnd < 512:
                            nc.scalar.activation(
                                out=aT[:, :, bnd:512], in_=sps[:, :, bnd:512],
                                func=AF.Exp, scale=-CSCALE,
                            )
                        for h in range(4):
                            nc.tensor.matmul(
                                o_ps[sh][32 * h : 32 * h + 32, :],
                                v_sb[:, t, 32 * h : 32 * h + 32],
                                aT[:, h, :],
                                start=(t == 0),
                                stop=(t == 7),
                                tile_position=(0, 32 * h),
                                skip_group_check=True,
                            )

                # epilogue: compact-transpose, normalize, LN(lna)
                o_sb = work.tile([128, S], f32, name=f"osb_{li}_{b}", tag="osb")
                for sh in range(2):
                    nc.vector.tensor_copy(o_sb[:, 512 * sh : 512 * (sh + 1)], o_ps[sh])
                T_ps = pp_sm.tile([128, 8, 20], f32, name=f"Tps_{li}_{b}", tag="psmall")
                for j in range(8):
                    nc.tensor.matmul(
                        T_ps[:, j, :],
                        o_sb[:, 128 * j : 128 * (j + 1)],
                        C["selT"],
                        start=True,
                        stop=True,
                    )
                T4 = T_ps.rearrange("p j (h c) -> p j h c", h=4)
                r_sb = small.tile([128, 8, 4], f32, name=f"r_{li}_{b}", tag="recip")
                nc.vector.reciprocal(out=r_sb, in_=T4[:, :, :, 4])
                araw = small.tile([128, 8, 4, 4], f32, name=f"araw_{li}_{b}", tag="araw")
                nc.vector.tensor_tensor(
                    out=araw, in0=T4[:, :, :, 0:4], in1=_brd(r_sb, 4), op=ALU.mult
                )
                araw2 = araw.rearrange("p j h c -> p (j h c)")
                anorm = ln_fold(araw2, f"lna{pref}_l")

                if li == 0:
                    att1 = ln_fold(anorm, "ln1_l", extra_add=xsrc_fold_b[b])
                    att1p = perb.tile([128, 128], f32, name=f"att1_{b}", tag="attf")
                    nc.vector.tensor_copy(att1p, att1)
                    att_fold_b[b] = att1p
                else:
                    att2 = ln_fold(anorm, "ln2_l")
                    # pooling + classifier
                    att2p = small.tile([128, 128], f32, name=f"att2_{b}", tag="att2")
                    nc.vector.tensor_copy(att2p, att2)
                    cs_ps = pp_sm.tile([128, 1], f32, name=f"cs_{b}", tag="psmall")
                    nc.tensor.matmul(cs_ps, att2p, C["ones_col"], start=True, stop=True)
                    cs_sb = small.tile([128, 1], f32, name=f"cssb_{b}", tag="cssb")
                    nc.vector.tensor_copy(cs_sb, cs_ps)
                    pl_ps = pp_sm.tile([16, 1], f32, name=f"pl_{b}", tag="psmall")
                    nc.tensor.matmul(pl_ps, C["selE"], cs_sb, start=True, stop=True)
                    pl_sb = small.tile([16, 1], f32, name=f"plsb_{b}", tag="plsb")
                    nc.vector.tensor_copy(pl_sb, pl_ps)
                    o10_ps = pp_sm.tile([1, NCLS], f32, name=f"o10_{b}", tag="psmall")
                    nc.tensor.matmul(o10_ps, pl_sb, C["woutT"], start=True, stop=True)
                    o10 = small.tile([1, NCLS], f32, name=f"o10sb_{b}", tag="o10")
                    nc.vector.tensor_add(o10, o10_ps, C["bout"])
                    nc.sync.dma_start(out=out_d[b], in_=o10)
        ctx.close()

    nc.compile()
    return nc


def _get_nc(ln_trivial_key):
    key = tuple(sorted(ln_trivial_key.items()))
    if key not in _BUILD_CACHE:
        _BUILD_CACHE[key] = build_nc(ln_trivial_key)
    return _BUILD_CACHE[key]


# ------------------------------------------------------------------ runner
def _run(inputs, trace=False, **kw):
    import ml_dtypes
    from concourse import bass_utils

    p = host_prep(inputs)
    nc = _get_nc(p["_ln_trivial"])

    base = {}
    for name, shape, isbf in CONST_SPECS:
        a = p[name].astype(ml_dtypes.bfloat16 if isbf is True else np.float32)
        assert a.shape == shape, (name, a.shape, shape)
        base[name] = a
    in_maps = []
    for c in range(NCORES):
        m = dict(base)
        m["x8"] = np.ascontiguousarray(p["x8"][NB * c : NB * (c + 1)]).astype(ml_dtypes.bfloat16)
        in_maps.append(m)

    res = bass_utils.run_bass_kernel_spmd(
        nc, in_maps, core_ids=list(range(NCORES)), trace=trace, **kw
    )
    out = np.concatenate(
        [res.results[c]["out"] for c in range(NCORES)], axis=0
    ).astype(np.float32)
    return out, res


def kernel(**inputs) -> np.ndarray:
    out, _ = _run(inputs, trace=False)
    return out


if __name__ == "__main__":
    import reference

    inputs = {k: np.asarray(v) for k, v in reference.setup_inputs().items()}
    import jax

    expected = np.asarray(reference.reference(**inputs))
    got = kernel(**inputs)
    err = np.abs(got - expected).max()
    rel = err / np.abs(expected).max()
    print("max abs err:", err, "rel:", rel)
